# revision 6
# baseline (speedup 1.0000x reference)
"""MelSpectrogram + PCEN Trainium2 kernel v2 (8-core data parallel).

Pipeline per core (8 batch elements):
  host: reflect-pad, hop-block transpose (512 x 2528), fp16 cast
  DVE:  u+/- = x_t +/- x_{t+2}  (A-step folded into DFT input)
  PE:   hop-block DFT via matmul -> A tiles directly (E fp16, 1/16-scaled)
  ACT:  PSUM->SBUF evac (f32->f16)
  DVE/GPSIMD: X-step (4-block phase combine, plane-swap adds only)
  DVE:  h = 0.5 x; wconv tmp/xw as two big flat subs (plane-major layout)
  PE:   q+-1 cross-plane shift tiles via shift-matrix matmuls
  DVE/ACT: square
  PE:   mel projection (fb folded with comp-duplication + s + scale)
  DVE:  PCEN IIR via tensor_tensor_scan (fp32 state)
  ACT:  PCEN pointwise ln/exp chain, per batch element, overlapped

Plane-major f-slot layout per comp c (r=0: cos, i=1: -sin), 9 tiles of 128:
  tiles 0-1: p0 (f=4q,   q=0..255)   tiles 2-3: p1 (f=4q+1)
  tiles 4-5: p2 (f=4q+2)             tiles 6-7: p3 (f=4q+3)
  tile  8:   px (slot 0 = f=1024; slots 1..127 zero)
comp i tiles are offset by 9. Compact 16-tile layout (no px) for h/tmp/pw.
"""

import math
from contextlib import ExitStack

import numpy as np

SR, N_FFT, HOP, N_MELS = 32000, 2048, 512, 128
F_MIN, F_MAX = 20.0, 16000.0
EPS, S, ALPHA, DELTA, R = 1e-6, 0.025, 0.98, 2.0, 0.5
NBINS = N_FFT // 2 + 1
T = 313
SBLK = 316
PAD = N_FFT // 2
B_TOTAL, L_WAVE = 64, 160000
N_CORES = 8

SC = 16.0    # E scale (E = E_true/SC)
SCM = 16.0   # mel scale
SCE = 256.0  # e2 scale (keeps (eps+m)^-alpha comfortably in range)
SCL = 8.0    # LT scale (keeps fp16 LT entries in normal range)
W = 512

# PCEN Toeplitz tiling: three overlapping 128-frame transpose tiles;
# each LT tile only "owns" the tau rows in its responsibility range.
LT_TILES = [(0, 0, 128), (128, 128, 256), (185, 256, 313)]  # (t0, lo, hi)


def _slot_of(f, c):
    if f == 1024:
        return c * 1152 + 1024
    p, q = f % 4, f // 4
    return c * 1152 + p * 256 + q


def _mel_fbank():
    def hz2mel(f):
        return 2595.0 * np.log10(1.0 + np.asarray(f, np.float64) / 700.0)

    def mel2hz(m):
        return 700.0 * (10.0 ** (np.asarray(m, np.float64) / 2595.0) - 1.0)

    all_freqs = np.linspace(0.0, SR / 2.0, NBINS)
    m_pts = np.linspace(hz2mel(F_MIN), hz2mel(F_MAX), N_MELS + 2)
    f_pts = mel2hz(m_pts)
    f_diff = np.diff(f_pts)
    slopes = f_pts[None, :] - all_freqs[:, None]
    down = -slopes[:, :-2] / f_diff[:-1]
    up = slopes[:, 2:] / f_diff[1:]
    return np.maximum(0.0, np.minimum(down, up))


def _build_consts():
    r = np.arange(HOP)
    E = np.zeros((HOP, 2304), np.float64)
    for f in range(NBINS):
        th = 2.0 * np.pi * f * r / N_FFT
        E[:, _slot_of(f, 0)] = np.cos(th) / SC
        E[:, _slot_of(f, 1)] = -np.sin(th) / SC
    fb = _mel_fbank()
    assert abs(fb[1024]).max() < 1e-9
    fb2 = np.zeros((2048, N_MELS), np.float64)
    for f in range(1024):
        wgt = fb[f] * (SC * SC / 4.0) * S / SCM
        for c in range(2):
            p, q = f % 4, f // 4
            fb2[c * 1024 + p * 256 + q] = wgt
    # LT[j][tau_local, t] = (1-S)^(t - tau) * SCL for tau in the tile's
    # responsibility range [lo, hi) and tau <= t (s itself is folded into
    # fb2, so melb = s*mel/SCM and msp = SCL*m/SCM).
    t = np.arange(T)
    lt = np.zeros((3, 128, T), np.float64)
    for j, (t0, lo, hi) in enumerate(LT_TILES):
        for tau in range(lo, hi):
            m = t >= tau
            lt[j, tau - t0, m] = (1.0 - S) ** (t[m] - tau) * SCL
    sdn = 0.5 * np.eye(128, k=1)
    cfirst = np.zeros((128, 128)); cfirst[127, 0] = 0.5
    sup = 0.5 * np.eye(128, k=-1)
    clast = np.zeros((128, 128)); clast[0, 127] = 0.5
    sh = np.concatenate([sdn, cfirst, sup, clast], axis=1)
    return E, fb2, lt, sh


def _make_chunks(NC):
    chunks = []
    co = 0
    while co < NC - 3:
        rem = NC - co
        if rem <= W:
            if rem > 352:
                chunks.append((co, 316)); co += 313
            else:
                chunks.append((co, rem)); co += rem - 3
        else:
            chunks.append((co, W)); co += W - 3
    return chunks


def _split_multiwaits(nc, limit=1):
    """This walrus build accepts at most `limit` sync-waits per instruction;
    move excess waits onto preceding same-engine NoOps."""
    import bass_rust
    import concourse.mybir as mybir

    for fn in nc.m.functions:
        for b in fn.blocks:
            insts = b.instructions
            new = []
            changed = False
            for i in insts:
                si = i.sync_info
                if si is not None and len(si.on_wait) > limit:
                    waits = list(si.on_wait)
                    for k in range(0, len(waits) - limit, limit):
                        chunk = waits[k : k + limit]
                        nop = mybir.InstNoOp(
                            name=f"{i.name}-wsplit{k}", ins=[], outs=[]
                        )
                        nop.engine = i.engine
                        nop.sync_info = bass_rust.SyncInfo(
                            on_wait=chunk, on_update=[]
                        )
                        new.append(nop)
                        changed = True
                    si.on_wait = waits[len(waits) - limit :]
                new.append(i)
            if changed:
                b.instructions = new


def build_nc(BL=8, split=True):
    import concourse.bass as bass
    import concourse.mybir as mybir
    from concourse import tile

    f16 = mybir.dt.float16
    f32 = mybir.dt.float32
    ALU = mybir.AluOpType
    ACTF = mybir.ActivationFunctionType

    NC = BL * SBLK
    chunks = _make_chunks(NC)
    ncb = len(chunks)

    nc = bass.Bass("TRN2", target_bir_lowering=False, debug=False)
    xt_d = nc.dram_tensor("xt", [4, 128, NC], f16, kind="ExternalInput")
    e_d = nc.dram_tensor("e", [4, 128, 2304], f16, kind="ExternalInput")
    fb_d = nc.dram_tensor("fb", [128, 2048], f16, kind="ExternalInput")
    lt_d = nc.dram_tensor("lt", [3, 128, T], f16, kind="ExternalInput")
    sh_d = nc.dram_tensor("sh", [128, 512], f16, kind="ExternalInput")
    y_d = nc.dram_tensor("y", [BL, 128, T], f32, kind="ExternalOutput")

    W18 = 18 * W
    W16 = 16 * W

    with tile.TileContext(nc) as tc, ExitStack() as top:
        cpool = top.enter_context(tc.tile_pool(name="consts", bufs=1))
        xb = cpool.tile([128, 4 * NC], f16)
        eb = cpool.tile([128, 4 * 2304], f16)
        fbb = cpool.tile([128, 2048], f16)
        melb = cpool.tile([128, NC], f16)
        ltb = cpool.tile([128, 3 * T], f16)
        shb = cpool.tile([128, 512], f16)

        xbv = xb[:, :].rearrange("p (rc c) -> p rc c", rc=4)
        ebv = eb[:, :].rearrange("p (rc c) -> p rc c", rc=4)
        ltv = ltb[:, :].rearrange("p (k t) -> p k t", k=3)
        shv = shb[:, :].rearrange("p (k c) -> p k c", k=4)
        # startup-latency ordering: small head slices first so dft(0) can
        # begin while the bulk still streams in
        EH, XH = 512, min(768, NC)
        for rc in range(4):
            nc.sync.dma_start(
                ebv[:, rc, 0:EH], e_d.ap()[rc][:, 0:EH]
            )
        for rc in range(4):
            nc.sync.dma_start(xbv[:, rc, 0:XH], xt_d.ap()[rc][:, 0:XH])
        for rc in range(4):
            nc.sync.dma_start(
                ebv[:, rc, EH:2304], e_d.ap()[rc][:, EH:2304]
            )
        nc.sync.dma_start(fbb[:, :], fb_d.ap()[:, :])
        nc.sync.dma_start(shb[:, :], sh_d.ap()[:, :])
        if XH < NC:
            for rc in range(4):
                nc.sync.dma_start(xbv[:, rc, XH:NC], xt_d.ap()[rc][:, XH:NC])
        for k in range(3):
            nc.sync.dma_start(ltv[:, k, :], lt_d.ap()[k])
        bias_t = cpool.tile([128, 4], f32)
        nc.vector.memset(bias_t[:, 0:1], EPS)
        nc.vector.memset(bias_t[:, 1:2], math.log(SCE))
        nc.vector.memset(bias_t[:, 2:3], DELTA)
        nc.vector.memset(bias_t[:, 3:4], 0.0)

        with ExitStack() as cph:
            # PSUM budget (8 banks): DFT ft-pairs 2x[128,2W] (4) +
            # xs shift singles 2x[128,W] (2) + mel/msp 2x[128,W] (2)
            yps = cph.enter_context(tc.tile_pool(name="yps", bufs=2, space="PSUM"))
            xsps = cph.enter_context(tc.tile_pool(name="xsps", bufs=2, space="PSUM"))
            mps = cph.enter_context(tc.tile_pool(name="mps", bufs=2, space="PSUM"))
            p_u = cph.enter_context(tc.tile_pool(name="p_u", bufs=2))
            p_a = cph.enter_context(tc.tile_pool(name="p_a", bufs=2))
            p_x = cph.enter_context(tc.tile_pool(name="p_x", bufs=1))
            p_h = cph.enter_context(tc.tile_pool(name="p_h", bufs=1))
            p_t = cph.enter_context(tc.tile_pool(name="p_t", bufs=1))
            p_pw = cph.enter_context(tc.tile_pool(name="p_pw", bufs=2))
            p_mt = cph.enter_context(tc.tile_pool(name="p_mt", bufs=2))
            p_ec = cph.enter_context(tc.tile_pool(name="p_ec", bufs=4))
            p_out = cph.enter_context(tc.tile_pool(name="p_out", bufs=2))

            us = [None] * ncb
            asbs = [None] * ncb

            def emit_u(cj):
                co, w = chunks[cj]
                u = p_u.tile([128, 2, 4, W], f16, tag="u")
                n2 = min(w, NC - co - 2)
                if n2 < W:
                    nc.gpsimd.memset(u[:, :, :, n2:W], 0.0)
                nc.vector.tensor_add(
                    u[:, 0, :, 0:n2], xbv[:, :, co : co + n2],
                    xbv[:, :, co + 2 : co + 2 + n2],
                )
                nc.vector.tensor_sub(
                    u[:, 1, :, 0:n2], xbv[:, :, co : co + n2],
                    xbv[:, :, co + 2 : co + 2 + n2],
                )
                us[cj] = u

            def emit_dft(cj):
                co, w = chunks[cj]
                u = us[cj]
                asb = p_a.tile([128, W18 + 8], f16, tag="asb")
                nc.gpsimd.memset(asb[:, W18 : W18 + 8], 0.0)
                if w < W:
                    # zero all 18 tile col-tails in one strided memset
                    asv = asb[:, 0:W18].rearrange("p (t c) -> p t c", t=18)
                    nc.gpsimd.memset(asv[:, :, w:W], 0.0)
                # ft pairs share one 2-bank PSUM tile -> one evac per pair
                for fp in range(9):
                    yp = yps.tile([128, 2 * W], f32, tag="yp")
                    for half in range(2):
                        ft = 2 * fp + half
                        c, tl = divmod(ft, 9)
                        usel = 0 if (tl >= 8 or (tl // 2) % 2 == 0) else 1
                        for rc in range(4):
                            nc.tensor.matmul(
                                yp[:, half * W : half * W + w],
                                ebv[:, rc, ft * 128 : (ft + 1) * 128],
                                u[:, usel, rc, 0:w],
                                start=(rc == 0),
                                stop=(rc == 3),
                            )
                    if w < W:
                        dst = asb[:, 2 * fp * W : (2 * fp + 2) * W].rearrange(
                            "p (t c) -> p t c", t=2
                        )[:, :, 0:w]
                        src = yp[:, :].rearrange("p (t c) -> p t c", t=2)[
                            :, :, 0:w
                        ]
                        nc.scalar.copy(dst, src)
                    else:
                        nc.scalar.copy(
                            asb[:, 2 * fp * W : (2 * fp + 2) * W], yp[:, :]
                        )
                asbs[cj] = asb

            def emit_X(ci):
                asb = asbs[ci]
                x = p_x.tile([128, W18 + 8], f16, tag="x")
                av = asb[:, 0:W18].rearrange("p (c t) -> p c t", c=2)
                ashv = asb[:, 1 : W18 + 1].rearrange("p (c t) -> p c t", c=2)
                xv = x[:, 0:W18].rearrange("p (c t) -> p c t", c=2)
                # p0 (+), p2 (-), px (+) on gpsimd
                nc.gpsimd.tensor_add(
                    xv[:, :, 0 : 2 * W], av[:, :, 0 : 2 * W], ashv[:, :, 0 : 2 * W]
                )
                nc.gpsimd.tensor_sub(
                    xv[:, :, 4 * W : 6 * W], av[:, :, 4 * W : 6 * W],
                    ashv[:, :, 4 * W : 6 * W],
                )
                nc.gpsimd.tensor_add(
                    xv[:, :, 8 * W : 9 * W], av[:, :, 8 * W : 9 * W],
                    ashv[:, :, 8 * W : 9 * W],
                )
                # p1: r = A_r + A_i[t+1]; i = A_i - A_r[t+1]
                O = 9 * W
                nc.vector.tensor_add(
                    x[:, 2 * W : 4 * W], asb[:, 2 * W : 4 * W],
                    asb[:, O + 2 * W + 1 : O + 4 * W + 1],
                )
                nc.vector.tensor_sub(
                    x[:, O + 2 * W : O + 4 * W], asb[:, O + 2 * W : O + 4 * W],
                    asb[:, 2 * W + 1 : 4 * W + 1],
                )
                # p3: r = A_r - A_i[t+1]; i = A_i + A_r[t+1]
                nc.vector.tensor_sub(
                    x[:, 6 * W : 8 * W], asb[:, 6 * W : 8 * W],
                    asb[:, O + 6 * W + 1 : O + 8 * W + 1],
                )
                nc.vector.tensor_add(
                    x[:, O + 6 * W : O + 8 * W], asb[:, O + 6 * W : O + 8 * W],
                    asb[:, 6 * W + 1 : 8 * W + 1],
                )
                return x

            def emit_xs(ci, x):
                """q+-1 shift tiles via PE shift-matmuls into paired PSUM,
                then ACT-evac into the hcat staging slots so tmp/xw become
                single flat DVE ops.

                hcat layout per comp (12W):
                  [xs3h(2W) | h_p0 | h_p1 | h_p2 | h_p3 | xs0h(2W)]
                """
                xv = x[:, 0:W18].rearrange("p (c t) -> p c t", c=2)
                hc = p_h.tile([128, 2 * 12 * W], f16, tag="hc")
                hv = hc[:, :].rearrange("p (c t) -> p c t", c=2)
                for c in range(2):
                    t3a = xsps.tile([128, W], f32, tag="xs")
                    nc.tensor.matmul(t3a[:, :], shv[:, 0, :],
                                     xv[:, c, 6 * W : 7 * W],
                                     start=True, stop=True)
                    nc.scalar.copy(hv[:, c, 0:W], t3a[:, :])
                    t3b = xsps.tile([128, W], f32, tag="xs")
                    nc.tensor.matmul(t3b[:, :], shv[:, 0, :],
                                     xv[:, c, 7 * W : 8 * W],
                                     start=True, stop=False)
                    nc.tensor.matmul(t3b[:, :], shv[:, 1, :],
                                     xv[:, c, 6 * W : 7 * W],
                                     start=False, stop=True)
                    nc.scalar.copy(hv[:, c, W : 2 * W], t3b[:, :])
                    t0a = xsps.tile([128, W], f32, tag="xs")
                    nc.tensor.matmul(t0a[:, :], shv[:, 2, :],
                                     xv[:, c, 0:W],
                                     start=True, stop=False)
                    nc.tensor.matmul(t0a[:, :], shv[:, 3, :],
                                     xv[:, c, W : 2 * W],
                                     start=False, stop=True)
                    nc.scalar.copy(hv[:, c, 10 * W : 11 * W], t0a[:, :])
                    t0b = xsps.tile([128, W], f32, tag="xs")
                    nc.tensor.matmul(t0b[:, :], shv[:, 2, :],
                                     xv[:, c, W : 2 * W],
                                     start=True, stop=False)
                    nc.tensor.matmul(t0b[:, :], shv[:, 3, :],
                                     xv[:, c, 8 * W : 9 * W],
                                     start=False, stop=True)
                    nc.scalar.copy(hv[:, c, 11 * W : 12 * W], t0b[:, :])
                return hc

            def emit_wconv(ci, x, hc):
                xv = x[:, 0:W18].rearrange("p (c t) -> p c t", c=2)
                hv = hc[:, :].rearrange("p (c t) -> p c t", c=2)
                # h main: 0.5*x planes p0..p3 into the middle slots
                nc.vector.tensor_scalar_mul(
                    hv[:, :, 2 * W : 10 * W], xv[:, :, 0 : 8 * W], 0.5
                )
                tmp = p_t.tile([128, W16], f16, tag="tmp")
                tv = tmp[:, :].rearrange("p (c t) -> p c t", c=2)
                # tmp_p = x_p - [xs3h, h_p0, h_p1, h_p2][p]
                nc.vector.tensor_sub(
                    tv[:, :, 0 : 8 * W], xv[:, :, 0 : 8 * W], hv[:, :, 0 : 8 * W]
                )
                # xw_p = tmp_p - [h_p1, h_p2, h_p3, xs0h][p]  (in place)
                nc.vector.tensor_sub(
                    tv[:, :, 0 : 8 * W], tv[:, :, 0 : 8 * W],
                    hv[:, :, 4 * W : 12 * W],
                )
                return tmp

            def emit_sq(ci, tmp):
                pw = p_pw.tile([128, W16], f16, tag="pw")
                half = 8 * W
                nc.vector.tensor_mul(
                    pw[:, 0:half], tmp[:, 0:half], tmp[:, 0:half]
                )
                nc.scalar.activation(pw[:, half:W16], tmp[:, half:W16], ACTF.Square)
                return pw

            def emit_mel(ci, pw):
                co, w = chunks[ci]
                V = min(w - 3, NC - 3 - co)
                mp = mps.tile([128, W], f32, tag="mp")
                for ct in range(16):
                    nc.tensor.matmul(
                        mp[:, 0:V],
                        fbb[:, ct * 128 : (ct + 1) * 128],
                        pw[:, ct * W : ct * W + V],
                        start=(ct == 0),
                        stop=(ct == 15),
                    )
                nc.scalar.copy(melb[:, co : co + V], mp[:, 0:V])

            # tail jobs: which b's mel is complete after chunk ci
            ready_after = [[] for _ in range(ncb)]
            for b in range(BL):
                need = b * SBLK + T
                for ci, (co, w) in enumerate(chunks):
                    V = min(w - 3, NC - 3 - co)
                    if co + V >= need:
                        ready_after[ci].append(b)
                        break

            def emit_tail(b):
                mv = melb[:, b * SBLK : b * SBLK + T]
                c1 = p_ec.tile([128, T], f32, tag="ec")
                c2 = p_ec.tile([128, T], f32, tag="ec")
                # PCEN smoother as lower-triangular Toeplitz matmul:
                # transpose melb (time onto partitions) via DMA XBAR, then
                # 3 accumulating matmuls against responsibility-masked LT.
                melT = p_mt.tile([128, 3 * 128], f16, tag="mt")
                for j, (t0, lo, hi) in enumerate(LT_TILES):
                    nc.sync.dma_start_transpose(
                        melT[:, j * 128 : (j + 1) * 128],
                        melb[:, b * SBLK + t0 : b * SBLK + t0 + 128],
                    )
                msp = mps.tile([128, W], f32, tag="mp")
                for j in range(3):
                    nc.tensor.matmul(
                        msp[:, 0:T],
                        melT[:, j * 128 : (j + 1) * 128],
                        ltv[:, j, :],
                        start=(j == 0),
                        stop=(j == 2),
                    )
                # e1 = ln(msp*(SCM/SCL) + EPS)
                nc.scalar.activation(
                    c1[:, :], msp[:, 0:T], ACTF.Ln, bias=bias_t[:, 0:1],
                    scale=SCM / SCL,
                )
                # e2 = exp(-alpha*e1 + ln(SCE)) = SCE*(eps+m)^-alpha
                nc.scalar.activation(
                    c2[:, :], c1[:, :], ACTF.Exp, bias=bias_t[:, 1:2], scale=-ALPHA
                )
                # e3 = e2 * melb  (melb = s*mel/SCM)
                nc.gpsimd.tensor_mul(c1[:, :], c2[:, :], mv)
                # e4 = ln(e3*(SCM/(SCE*s)) + DELTA)
                nc.scalar.activation(
                    c2[:, :], c1[:, :], ACTF.Ln, bias=bias_t[:, 2:3],
                    scale=SCM / (SCE * S),
                )
                # e5 = exp(R*e4); out = e5 - DELTA^R
                nc.scalar.activation(
                    c1[:, :], c2[:, :], ACTF.Exp, bias=bias_t[:, 3:4], scale=R
                )
                ob = p_out.tile([128, T], f32, tag="ob")
                nc.vector.tensor_scalar_add(ob[:, :], c1[:, :], -(DELTA**R))
                nc.sync.dma_start(y_d.ap()[b], ob[:, :])

            # ---- pipelined emission ----
            emit_u(0)
            if ncb > 1:
                emit_u(1)
            emit_dft(0)
            if ncb > 1:
                emit_dft(1)
            pending_tails = []
            for ci in range(ncb):
                if ci + 2 < ncb:
                    emit_u(ci + 2)
                x = emit_X(ci)
                asbs[ci] = None
                hc = emit_xs(ci, x)
                tmp = emit_wconv(ci, x, hc)
                pw = emit_sq(ci, tmp)
                if ci + 2 < ncb:
                    emit_dft(ci + 2)
                emit_mel(ci, pw)
                for b in pending_tails:
                    emit_tail(b)
                pending_tails = ready_after[ci]
            for b in pending_tails:
                emit_tail(b)

    if split:
        _split_multiwaits(nc)
    return nc


# ---------------------------------------------------------------- host side

_CACHE = {}


def _get_consts():
    if "consts" not in _CACHE:
        E, fb2, lt, sh = _build_consts()
        e_h = np.ascontiguousarray(
            E.astype(np.float16).reshape(4, 128, 2304), dtype=np.float16
        )
        # fb tile layout: fb_h[p, ct*128+m] = fb2[ct*128+p, m]
        fb_h = np.ascontiguousarray(
            fb2.astype(np.float16).reshape(16, 128, 128).transpose(1, 0, 2)
            .reshape(128, 2048)
        )
        lt_h = np.ascontiguousarray(lt.astype(np.float16))
        sh_h = np.ascontiguousarray(sh.astype(np.float16))
        _CACHE["consts"] = (e_h, fb_h, lt_h, sh_h)
    return _CACHE["consts"]


def _prep_core_input(wf_core):
    """wf_core: [BL, 160000] f32 -> xt [4, 128, BL*316] f16."""
    BL = wf_core.shape[0]
    x = np.pad(wf_core, ((0, 0), (PAD, PAD)), mode="reflect")
    blocks = x[:, : SBLK * HOP].reshape(BL, SBLK, HOP)
    xT = blocks.transpose(2, 0, 1).reshape(HOP, BL * SBLK)
    return np.ascontiguousarray(
        xT.astype(np.float16).reshape(4, 128, BL * SBLK)
    )


def _build_in_maps(waveform):
    e_h, fb_h, lt_h, sh_h = _get_consts()
    BL = B_TOTAL // N_CORES
    in_maps = []
    for c in range(N_CORES):
        xt = _prep_core_input(waveform[c * BL : (c + 1) * BL])
        in_maps.append(
            {"xt": xt, "e": e_h, "fb": fb_h, "lt": lt_h, "sh": sh_h}
        )
    return in_maps


def _get_nc():
    if "nc" not in _CACHE:
        _CACHE["nc"] = build_nc(BL=8)
    return _CACHE["nc"]


def kernel(waveform: np.ndarray) -> np.ndarray:
    from concourse.bass_utils import run_bass_kernel_spmd

    waveform = np.asarray(waveform, np.float32)
    assert waveform.shape == (B_TOTAL, L_WAVE)
    in_maps = _build_in_maps(waveform)
    nc = _get_nc()
    res = run_bass_kernel_spmd(nc, in_maps, core_ids=list(range(N_CORES)))
    BL = B_TOTAL // N_CORES
    out = np.empty((B_TOTAL, 1, N_MELS, T), np.float32)
    for c in range(N_CORES):
        y = np.asarray(res.results[c]["y"])  # [BL, 128, T]
        out[c * BL : (c + 1) * BL, 0] = y
    return out
